# revision 18
# baseline (speedup 1.0000x reference)
"""GPT2 attention (B=4, S=2048, D=768, H=12, no causal mask) on 8 trn2 cores.

Sharding: core c -> batch b = c//2, head-group g = c%2 (6 heads of 64).
Each core computes its 6 heads' attention + the matching row-block of the
output projection; host sums the two per-batch partials and adds b_proj.

v2 design (ACT-bound pipeline, target ~230 us/core):
  - x is transposed on HOST -> xT [768, 2048] bf16 (no PE transposes).
  - q-scale 1/sqrt(64) folded into Wq/bq on host.
  - scores: head-pair packed via PE row tiling (K=64, tile_position
    (0,0)/(64,0)) -> psum [128 sk, 1024] covering TWO sk-chunks per head,
    exp'd by ONE ACT instruction (FD=1024, cuts ACT overhead ~19%).
  - attn@v: vaug [sk, 64 v | 64 ones] accumulating [128, 512] psum; row 64+
    is the softmax denominator. Normalize: DVE reciprocal + multiply.
  - proj: attnT is the natural lhsT; DVE copy psum->SBUF, DMA out fp32.
  - Emission interleaving: ACT (exp) is the bottleneck engine (~215 us);
    qkv/v/proj matmul "filler" units are generator-paced into the attention
    group stream so the PE never idles long (HAM stays warm) and ACT is
    saturated from ~15 us onward.
"""

import json
from collections import deque
from contextlib import ExitStack

import ml_dtypes
import numpy as np

import concourse.bass as bass
import concourse.mybir as mybir
import concourse.tile as tile
from concourse.bass_utils import run_bass_kernel_spmd

B, S, D = 4, 2048, 768
H, HD = 12, 64
HPC = 6            # heads per core
DKC = HPC * HD     # 384: per-core width of q/k/v
NPAIR = HPC // 2   # 3 head pairs
P = 128
F32 = mybir.dt.float32
BF16 = mybir.dt.bfloat16

NSQ = S // 512     # 4 sq blocks
NST = S // 128     # 16 sk chunks
NDC = D // 128     # 6 d chunks
NG = NST // 2      # 8 chunk groups (2 sk chunks per exp)


def _split_multi_waits(bir_bytes):
    """Walrus in this toolchain accepts only one sync-wait per instruction.

    Hoist extra waits onto same-engine NoOps inserted just before. Engines
    execute their stream in order and semaphores are monotonic, so this is
    semantically identical.
    """
    m = json.loads(bir_bytes)
    for fn in m["functions"]:
        for blk in fn["blocks"]:
            new = []
            for ins in blk["instructions"]:
                si = ins.get("sync_info")
                waits = (si or {}).get("on_wait") or []
                if len(waits) > 1:
                    for j, w in enumerate(waits[:-1]):
                        new.append({
                            "debug": ins.get("debug", 0),
                            "engine": ins["engine"],
                            "ins": [], "outs": [],
                            "name": f"{ins['name']}w{j}",
                            "opcode": "NoOp",
                            "sync_info": {"on_update": [], "on_wait": [w]},
                        })
                    si["on_wait"] = [waits[-1]]
                new.append(ins)
            blk["instructions"] = new
    return json.dumps(m).encode()


class FillQ:
    """Generator-paced PE filler work, interleaved into the attention stream."""

    def __init__(self):
        self.gens = deque()
        self.done = set()

    def push(self, gen):
        self.gens.append(gen)

    def _pull(self):
        while self.gens:
            try:
                m = next(self.gens[0])
                if m:
                    self.done.add(m)
                return True
            except StopIteration:
                self.gens.popleft()
        return False

    def step(self, n=1):
        for _ in range(n):
            if not self._pull():
                return

    def flush_until(self, marker):
        while marker not in self.done:
            if not self._pull():
                raise RuntimeError(f"filler queue exhausted before {marker}")

    def flush_all(self):
        while self._pull():
            pass

    def drive_all(self, gen):
        """Run a generator to exhaustion immediately (upfront work)."""
        for m in gen:
            if m:
                self.done.add(m)


def build_kernel():
    nc = bass.Bass("TRN2", target_bir_lowering=False, debug=False)
    xT_d = nc.dram_tensor("xT", [D, S], BF16, kind="ExternalInput").ap()
    wqkv_d = nc.dram_tensor("wqkv", [D, 3 * DKC], BF16, kind="ExternalInput").ap()
    bqkv_d = nc.dram_tensor("bqkv", [3 * DKC], F32, kind="ExternalInput").ap()
    wproj_d = nc.dram_tensor("wproj", [DKC, D], BF16, kind="ExternalInput").ap()
    out_d = nc.dram_tensor("out", [S, D], F32, kind="ExternalOutput").ap()

    with tile.TileContext(nc) as tc:
        with ExitStack() as ctx:
            _body(ctx, tc, xT_d, wqkv_d, bqkv_d, wproj_d, out_d)
    orig_to_json = nc.to_json_bytes
    nc.to_json_bytes = lambda: _split_multi_waits(orig_to_json())
    return nc


def _body(ctx, tc, xT_d, wqkv_d, bqkv_d, wproj_d, out_d):
    nc = tc.nc
    ADD = mybir.AluOpType.add
    MULT = mybir.AluOpType.mult
    EXP = mybir.ActivationFunctionType.Exp

    consts = ctx.enter_context(tc.tile_pool(name="consts", bufs=1))
    big = ctx.enter_context(tc.tile_pool(name="big", bufs=1))
    epool = ctx.enter_context(tc.tile_pool(name="epool", bufs=6))
    smalls = ctx.enter_context(tc.tile_pool(name="smalls", bufs=2))
    outst = ctx.enter_context(tc.tile_pool(name="outst", bufs=3))
    stage = ctx.enter_context(tc.tile_pool(name="stage", bufs=4))
    ps_sc = ctx.enter_context(tc.tile_pool(name="ps_sc", bufs=2, space="PSUM"))
    ps_acc = ctx.enter_context(tc.tile_pool(name="ps_acc", bufs=2, space="PSUM"))
    ps_misc = ctx.enter_context(tc.tile_pool(name="ps_misc", bufs=2, space="PSUM"))

    # --- constants / biases (tiny DMAs first: they gate qk bias adds) ---
    bq_sb = consts.tile([P, 3], F32)
    nc.sync.dma_start(out=bq_sb, in_=bqkv_d[0:DKC].rearrange("(t p) -> p t", p=P))
    bk_sb = consts.tile([P, 3], F32)
    nc.sync.dma_start(out=bk_sb, in_=bqkv_d[DKC:2 * DKC].rearrange("(t p) -> p t", p=P))
    bv_row = consts.tile([1, DKC], F32)
    nc.sync.dma_start(out=bv_row, in_=bqkv_d[2 * DKC:3 * DKC].rearrange("(o f) -> o f", o=1))
    bv16 = consts.tile([1, DKC], BF16)
    nc.vector.tensor_copy(out=bv16, in_=bv_row)
    ones_row = consts.tile([1, P], BF16)
    nc.vector.memset(ones_row, 1.0)

    # warm the ACT exp table during the ramp (one-time ~2.7us load)
    wsrc = consts.tile([1, 8], F32)
    nc.vector.memset(wsrc, 0.0)
    wdst = consts.tile([1, 8], F32)
    nc.scalar.activation(out=wdst, in_=wsrc, func=EXP)

    # --- weights + xT loads (interleaved so chunk-0 compute starts early) ---
    w_sb = []
    xT = []
    for c in range(NDC):
        t = big.tile([P, 3 * DKC], BF16, name=f"w_sb{c}")
        nc.sync.dma_start(out=t, in_=wqkv_d[c * P:(c + 1) * P, :])
        w_sb.append(t)
        t2 = big.tile([P, S], BF16, name=f"xT{c}")
        nc.sync.dma_start(out=t2, in_=xT_d[c * P:(c + 1) * P, :])
        xT.append(t2)
    wproj_sb = []
    for t3 in range(NPAIR):
        t = big.tile([P, D], BF16, name=f"wproj_sb{t3}")
        nc.sync.dma_start(out=t, in_=wproj_d[t3 * P:(t3 + 1) * P, :])
        wproj_sb.append(t)

    qT = [big.tile([P, S], BF16, name=f"qT{t}") for t in range(NPAIR)]
    kT = [big.tile([P, S], BF16, name=f"kT{t}") for t in range(NPAIR)]
    vaug = big.tile([P, HPC, NST, P], BF16, name="vaug")
    nc.vector.memset(vaug[:, :, :, 64:128], 1.0)
    attnT = [big.tile([P, S], BF16, name=f"attnT{t}") for t in range(NPAIR)]

    # --- filler generators -------------------------------------------------
    def g_v(sts):
        for st in sts:
            ps = ps_misc.tile([P, 512], F32, name="psv", tag="misc")
            for c in range(NDC):
                nc.tensor.matmul(
                    ps[:, 0:DKC],
                    lhsT=xT[c][:, st * P:(st + 1) * P],
                    rhs=w_sb[c][:, 2 * DKC:3 * DKC],
                    start=(c == 0), stop=False)
                if c == 2:
                    yield None
            nc.tensor.matmul(ps[:, 0:DKC], lhsT=ones_row, rhs=bv16,
                             start=False, stop=True)
            nc.vector.tensor_copy(
                out=vaug[:, :, st, 0:64],
                in_=ps[:, 0:DKC].rearrange("p (h e) -> p h e", h=HPC))
            yield f"v{st}"

    def g_qk(pr, which, sbs):
        # which: 0 = q, 1 = k
        dst = (qT if which == 0 else kT)[pr]
        bias = bq_sb if which == 0 else bk_sb
        for sb in sbs:
            ps = ps_misc.tile([P, 512], F32, name="psqk", tag="misc")
            for c in range(NDC):
                nc.tensor.matmul(
                    ps,
                    lhsT=w_sb[c][:, which * DKC + pr * P:which * DKC + (pr + 1) * P],
                    rhs=xT[c][:, sb * 512:(sb + 1) * 512],
                    start=(c == 0), stop=(c == NDC - 1))
                if c == 2:
                    yield None
            nc.vector.tensor_scalar(
                out=dst[:, sb * 512:(sb + 1) * 512], in0=ps,
                scalar1=bias[:, pr:pr + 1], scalar2=None, op0=ADD)
            yield (f"q{pr}_{sb}" if which == 0 else f"k{pr}_{sb}")

    def g_proj(sb):
        for i in range(4):
            st = sb * 4 + i
            ostg = outst.tile([P, D], F32, name="ostg", tag="ostg")
            for half in range(2):
                pp = ps_misc.tile([P, 512], F32, name="pp", tag="misc")
                for t3 in range(NPAIR):
                    nc.tensor.matmul(
                        pp[:, 0:384],
                        lhsT=attnT[t3][:, st * P:(st + 1) * P],
                        rhs=wproj_sb[t3][:, half * 384:(half + 1) * 384],
                        start=(t3 == 0), stop=(t3 == 2))
                yield None
                nc.vector.tensor_copy(out=ostg[:, half * 384:(half + 1) * 384],
                                      in_=pp[:, 0:384])
                yield None
            nc.sync.dma_start(out=out_d[st * P:(st + 1) * P, :], in_=ostg)
            yield None

    fq = FillQ()
    # upfront (cold clock, DMA-paced): only what attention chunk 0 needs --
    # kT(pair0) sb-chunk 0, qT(pair0, sb0), v chunks 0-5. The rest fills in.
    fq.drive_all(g_qk(0, 1, [0]))
    fq.drive_all(g_qk(0, 0, [0]))
    fq.drive_all(g_v(range(0, 6)))
    # paced fillers, ordered by first-need
    fq.push(g_v(range(6, 16)))
    fq.push(g_qk(0, 1, [1, 2, 3]))
    fq.push(g_qk(1, 1, [0, 1, 2, 3]))
    fq.push(g_qk(1, 0, [0]))
    fq.push(g_qk(0, 0, [1]))
    fq.push(g_qk(2, 1, [0, 1, 2, 3]))
    fq.push(g_qk(2, 0, [0]))
    fq.push(g_qk(0, 0, [2]))
    fq.push(g_qk(1, 0, [1]))
    fq.push(g_qk(2, 0, [1]))
    fq.push(g_qk(0, 0, [3]))
    fq.push(g_qk(1, 0, [2]))
    fq.push(g_qk(2, 0, [2]))
    fq.push(g_qk(1, 0, [3]))
    fq.push(g_qk(2, 0, [3]))

    # --- attention ---------------------------------------------------------
    # Score psum tiles are per sk-chunk and hold BOTH heads side by side
    # ([:, 0:512] = head A, [:, 512:1024] = head B, written by one
    # row-tiled concurrent matmul pair). One exp covers the pair (FD=1024).
    # With bufs=2 this gives a full chunk of lookahead: scores(ck+1) fill
    # while exp(ck) runs, so the ACT stream never waits on the PE refill.
    def emit_av(pr, prev, accA, accB):
        ck, e = prev
        st_f = (ck == 0)
        sp_f = (ck == NST - 1)
        nc.tensor.matmul(accA, lhsT=vaug[:, 2 * pr, ck, :],
                         rhs=e[:, 0:512], start=st_f, stop=sp_f)
        nc.tensor.matmul(accB, lhsT=vaug[:, 2 * pr + 1, ck, :],
                         rhs=e[:, 512:1024], start=st_f, stop=sp_f)

    # Deferred-normalize bookkeeping: the 2x3.3us DVE reciprocal burst for a
    # finished pair is emitted at chunk 1 of the NEXT pair -- a point where no
    # filler psum-ring consumer can queue behind it on the in-order DVE (the
    # boundary-stall / HAM-throttle trigger seen in traces).
    pending_norm = []

    def drain_norm():
        while pending_norm:
            pr_, sb_, stg_ = pending_norm.pop(0)
            for hh, st_t in enumerate(stg_):
                rec = smalls.tile([64, 512], F32, name="rec", tag="rec")
                nc.vector.reciprocal(out=rec, in_=st_t[64:128, :])
                nc.vector.tensor_tensor(
                    out=attnT[pr_][64 * hh:64 * (hh + 1),
                                   sb_ * 512:(sb_ + 1) * 512],
                    in0=st_t[0:64, :], in1=rec, op=MULT)

    for sb in range(NSQ):
        for pr in range(NPAIR):
            fq.flush_until(f"q{pr}_{sb}")
            accA = ps_acc.tile([P, 512], F32, name="accA", tag="acc")
            accB = ps_acc.tile([P, 512], F32, name="accB", tag="acc")
            prevs = []
            for ck in range(NST):
                fq.flush_until(f"k{pr}_{ck // 4}")
                fq.flush_until(f"v{ck}")
                if ck == 1:
                    drain_norm()
                # No fillers until the normalize burst has drained (ck>=8);
                # their misc-psum slot consumers (DVE) would queue behind it
                # and stall the in-order PE stream for many us.
                if ck >= 8 or (sb == 0 and pr == 0):
                    fq.step(2)
                sc = ps_sc.tile([P, 1024], F32, name="sc", tag="sc")
                nc.tensor.matmul(
                    sc[:, 0:512],
                    lhsT=kT[pr][0:64, ck * P:(ck + 1) * P],
                    rhs=qT[pr][0:64, sb * 512:(sb + 1) * 512],
                    start=True, stop=True, tile_position=(0, 0))
                nc.tensor.matmul(
                    sc[:, 512:1024],
                    lhsT=kT[pr][64:128, ck * P:(ck + 1) * P],
                    rhs=qT[pr][64:128, sb * 512:(sb + 1) * 512],
                    start=True, stop=True, tile_position=(64, 0))
                # attn@v runs two exps behind: the pair's first accumulator
                # write lands ~2.7us in, past the boundary DVE backlog that
                # releases the acc bank (avoids the in-order PE stall).
                if len(prevs) == 2:
                    emit_av(pr, prevs.pop(0), accA, accB)
                e = epool.tile([P, 1024], BF16, name="e", tag="e")
                nc.scalar.activation(out=e, in_=sc, func=EXP)
                prevs.append((ck, e))
            for pv in prevs:
                emit_av(pr, pv, accA, accB)

            # Copy-first normalize: a cheap FD-bound PSUM->SBUF copy frees the
            # acc bank in ~0.7us; the recip+mult burst is deferred to the
            # next pair's chunk 1 (see drain_norm above).
            # The copies run on the Scalar/ACT engine (PSUM-capable): +0.7us
            # each on the ACT stream, but the acc release no longer queues
            # behind the filler backlog on the in-order DVE.
            stg = []
            for acc in (accA, accB):
                st_t = stage.tile([P, 512], F32, name="stg", tag="stage")
                nc.scalar.copy(out=st_t, in_=acc)
                stg.append(st_t)
            pending_norm.append((pr, sb, stg))
        fq.push(g_proj(sb))
    drain_norm()
    fq.flush_all()


_NC_CACHE = None


def _get_nc():
    global _NC_CACHE
    if _NC_CACHE is None:
        _NC_CACHE = build_kernel()
    return _NC_CACHE


def make_in_maps(hidden_states, W_attn, b_attn, W_proj, b_proj):
    in_maps = []
    qscale = 1.0 / 8.0  # 1/sqrt(HD), folded into Wq/bq on host
    for c in range(8):
        b, g = c // 2, c % 2
        cols = slice(g * DKC, (g + 1) * DKC)
        wq = W_attn[:, 0 * D:1 * D][:, cols] * qscale
        wk = W_attn[:, 1 * D:2 * D][:, cols]
        wv = W_attn[:, 2 * D:3 * D][:, cols]
        bq = b_attn[0 * D:1 * D][cols] * qscale
        bk = b_attn[1 * D:2 * D][cols]
        bv = b_attn[2 * D:3 * D][cols]
        in_maps.append({
            "xT": np.ascontiguousarray(hidden_states[b].T).astype(ml_dtypes.bfloat16),
            "wqkv": np.ascontiguousarray(
                np.concatenate([wq, wk, wv], axis=1)).astype(ml_dtypes.bfloat16),
            "bqkv": np.ascontiguousarray(
                np.concatenate([bq, bk, bv]), dtype=np.float32),
            "wproj": np.ascontiguousarray(
                W_proj[g * DKC:(g + 1) * DKC, :]).astype(ml_dtypes.bfloat16),
        })
    return in_maps


def run(hidden_states, W_attn, b_attn, W_proj, b_proj, trace=False):
    nc = _get_nc()
    in_maps = make_in_maps(hidden_states, W_attn, b_attn, W_proj, b_proj)
    res = run_bass_kernel_spmd(nc, in_maps, core_ids=list(range(8)), trace=trace)
    out = np.empty((B, S, D), dtype=np.float32)
    for b in range(B):
        out[b] = res.results[2 * b]["out"] + res.results[2 * b + 1]["out"] + b_proj
    return out, res


def kernel(hidden_states, W_attn, b_attn, W_proj, b_proj):
    hidden_states = np.asarray(hidden_states, dtype=np.float32)
    W_attn = np.asarray(W_attn, dtype=np.float32)
    b_attn = np.asarray(b_attn, dtype=np.float32)
    W_proj = np.asarray(W_proj, dtype=np.float32)
    b_proj = np.asarray(b_proj, dtype=np.float32)
    out, _ = run(hidden_states, W_attn, b_attn, W_proj, b_proj, trace=False)
    return out


# revision 19
# speedup vs baseline: 1.0585x; 1.0585x over previous
"""GPT2 attention (B=4, S=2048, D=768, H=12, no causal mask) on 8 trn2 cores.

Sharding: core c -> batch b = c//2, head-group g = c%2 (6 heads of 64).
Each core computes its 6 heads' attention + the matching row-block of the
output projection; host sums the two per-batch partials and adds b_proj.

v2 design (ACT-bound pipeline, target ~230 us/core):
  - x is transposed on HOST -> xT [768, 2048] bf16 (no PE transposes).
  - q-scale 1/sqrt(64) folded into Wq/bq on host.
  - scores: head-pair packed via PE row tiling (K=64, tile_position
    (0,0)/(64,0)) -> psum [128 sk, 1024] covering TWO sk-chunks per head,
    exp'd by ONE ACT instruction (FD=1024, cuts ACT overhead ~19%).
  - attn@v: vaug [sk, 64 v | 64 ones] accumulating [128, 512] psum; row 64+
    is the softmax denominator. Normalize: DVE reciprocal + multiply.
  - proj: attnT is the natural lhsT; DVE copy psum->SBUF, DMA out fp32.
  - Emission interleaving: ACT (exp) is the bottleneck engine (~215 us);
    qkv/v/proj matmul "filler" units are generator-paced into the attention
    group stream so the PE never idles long (HAM stays warm) and ACT is
    saturated from ~15 us onward.
"""

import json
from collections import deque
from contextlib import ExitStack

import ml_dtypes
import numpy as np

import concourse.bass as bass
import concourse.mybir as mybir
import concourse.tile as tile
from concourse.bass_utils import run_bass_kernel_spmd

B, S, D = 4, 2048, 768
H, HD = 12, 64
HPC = 6            # heads per core
DKC = HPC * HD     # 384: per-core width of q/k/v
NPAIR = HPC // 2   # 3 head pairs
P = 128
F32 = mybir.dt.float32
BF16 = mybir.dt.bfloat16

NSQ = S // 512     # 4 sq blocks
NST = S // 128     # 16 sk chunks
NDC = D // 128     # 6 d chunks
NG = NST // 2      # 8 chunk groups (2 sk chunks per exp)


def _split_multi_waits(bir_bytes):
    """Walrus in this toolchain accepts only one sync-wait per instruction.

    Hoist extra waits onto same-engine NoOps inserted just before. Engines
    execute their stream in order and semaphores are monotonic, so this is
    semantically identical.
    """
    m = json.loads(bir_bytes)
    for fn in m["functions"]:
        for blk in fn["blocks"]:
            new = []
            for ins in blk["instructions"]:
                si = ins.get("sync_info")
                waits = (si or {}).get("on_wait") or []
                if len(waits) > 1:
                    for j, w in enumerate(waits[:-1]):
                        new.append({
                            "debug": ins.get("debug", 0),
                            "engine": ins["engine"],
                            "ins": [], "outs": [],
                            "name": f"{ins['name']}w{j}",
                            "opcode": "NoOp",
                            "sync_info": {"on_update": [], "on_wait": [w]},
                        })
                    si["on_wait"] = [waits[-1]]
                new.append(ins)
            blk["instructions"] = new
    return json.dumps(m).encode()


class FillQ:
    """Generator-paced PE filler work, interleaved into the attention stream."""

    def __init__(self):
        self.gens = deque()
        self.done = set()

    def push(self, gen):
        self.gens.append(gen)

    def _pull(self):
        while self.gens:
            try:
                m = next(self.gens[0])
                if m:
                    self.done.add(m)
                return True
            except StopIteration:
                self.gens.popleft()
        return False

    def step(self, n=1):
        for _ in range(n):
            if not self._pull():
                return

    def flush_until(self, marker):
        while marker not in self.done:
            if not self._pull():
                raise RuntimeError(f"filler queue exhausted before {marker}")

    def flush_all(self):
        while self._pull():
            pass

    def drive_all(self, gen):
        """Run a generator to exhaustion immediately (upfront work)."""
        for m in gen:
            if m:
                self.done.add(m)


def build_kernel():
    nc = bass.Bass("TRN2", target_bir_lowering=False, debug=False)
    xT_d = nc.dram_tensor("xT", [D, S], BF16, kind="ExternalInput").ap()
    wqkv_d = nc.dram_tensor("wqkv", [D, 3 * DKC], BF16, kind="ExternalInput").ap()
    bqkv_d = nc.dram_tensor("bqkv", [3 * DKC], F32, kind="ExternalInput").ap()
    wproj_d = nc.dram_tensor("wproj", [DKC, D], BF16, kind="ExternalInput").ap()
    out_d = nc.dram_tensor("out", [S, D], F32, kind="ExternalOutput").ap()

    with tile.TileContext(nc) as tc:
        with ExitStack() as ctx:
            _body(ctx, tc, xT_d, wqkv_d, bqkv_d, wproj_d, out_d)
    orig_to_json = nc.to_json_bytes
    nc.to_json_bytes = lambda: _split_multi_waits(orig_to_json())
    return nc


def _body(ctx, tc, xT_d, wqkv_d, bqkv_d, wproj_d, out_d):
    nc = tc.nc
    ADD = mybir.AluOpType.add
    MULT = mybir.AluOpType.mult
    EXP = mybir.ActivationFunctionType.Exp

    consts = ctx.enter_context(tc.tile_pool(name="consts", bufs=1))
    big = ctx.enter_context(tc.tile_pool(name="big", bufs=1))
    epool = ctx.enter_context(tc.tile_pool(name="epool", bufs=6))
    smalls = ctx.enter_context(tc.tile_pool(name="smalls", bufs=2))
    outst = ctx.enter_context(tc.tile_pool(name="outst", bufs=3))
    stage = ctx.enter_context(tc.tile_pool(name="stage", bufs=4))
    ps_sc = ctx.enter_context(tc.tile_pool(name="ps_sc", bufs=2, space="PSUM"))
    ps_acc = ctx.enter_context(tc.tile_pool(name="ps_acc", bufs=2, space="PSUM"))
    ps_misc = ctx.enter_context(tc.tile_pool(name="ps_misc", bufs=2, space="PSUM"))

    # --- constants / biases (tiny DMAs first: they gate qk bias adds) ---
    bq_sb = consts.tile([P, 3], F32)
    nc.sync.dma_start(out=bq_sb, in_=bqkv_d[0:DKC].rearrange("(t p) -> p t", p=P))
    bk_sb = consts.tile([P, 3], F32)
    nc.sync.dma_start(out=bk_sb, in_=bqkv_d[DKC:2 * DKC].rearrange("(t p) -> p t", p=P))
    bv_row = consts.tile([1, DKC], F32)
    nc.sync.dma_start(out=bv_row, in_=bqkv_d[2 * DKC:3 * DKC].rearrange("(o f) -> o f", o=1))
    bv16 = consts.tile([1, DKC], BF16)
    nc.vector.tensor_copy(out=bv16, in_=bv_row)
    ones_row = consts.tile([1, P], BF16)
    nc.vector.memset(ones_row, 1.0)

    # warm the ACT exp table during the ramp (one-time ~2.7us load)
    wsrc = consts.tile([1, 8], F32)
    nc.vector.memset(wsrc, 0.0)
    wdst = consts.tile([1, 8], F32)
    nc.scalar.activation(out=wdst, in_=wsrc, func=EXP)

    # --- weights + xT loads (interleaved so chunk-0 compute starts early) ---
    w_sb = []
    xT = []
    for c in range(NDC):
        t = big.tile([P, 3 * DKC], BF16, name=f"w_sb{c}")
        nc.sync.dma_start(out=t, in_=wqkv_d[c * P:(c + 1) * P, :])
        w_sb.append(t)
        t2 = big.tile([P, S], BF16, name=f"xT{c}")
        nc.sync.dma_start(out=t2, in_=xT_d[c * P:(c + 1) * P, :])
        xT.append(t2)
    wproj_sb = []
    for t3 in range(NPAIR):
        t = big.tile([P, D], BF16, name=f"wproj_sb{t3}")
        nc.sync.dma_start(out=t, in_=wproj_d[t3 * P:(t3 + 1) * P, :])
        wproj_sb.append(t)

    qT = [big.tile([P, S], BF16, name=f"qT{t}") for t in range(NPAIR)]
    kT = [big.tile([P, S], BF16, name=f"kT{t}") for t in range(NPAIR)]
    vaug = big.tile([P, HPC, NST, P], BF16, name="vaug")
    nc.vector.memset(vaug[:, :, :, 64:128], 1.0)
    attnT = [big.tile([P, S], BF16, name=f"attnT{t}") for t in range(NPAIR)]

    # --- filler generators -------------------------------------------------
    def g_v(sts):
        for st in sts:
            ps = ps_misc.tile([P, 512], F32, name="psv", tag="misc")
            for c in range(NDC):
                nc.tensor.matmul(
                    ps[:, 0:DKC],
                    lhsT=xT[c][:, st * P:(st + 1) * P],
                    rhs=w_sb[c][:, 2 * DKC:3 * DKC],
                    start=(c == 0), stop=False)
                if c == 2:
                    yield None
            nc.tensor.matmul(ps[:, 0:DKC], lhsT=ones_row, rhs=bv16,
                             start=False, stop=True)
            nc.vector.tensor_copy(
                out=vaug[:, :, st, 0:64],
                in_=ps[:, 0:DKC].rearrange("p (h e) -> p h e", h=HPC))
            yield f"v{st}"

    def g_qk(pr, which, sbs):
        # which: 0 = q, 1 = k
        dst = (qT if which == 0 else kT)[pr]
        bias = bq_sb if which == 0 else bk_sb
        for sb in sbs:
            ps = ps_misc.tile([P, 512], F32, name="psqk", tag="misc")
            for c in range(NDC):
                nc.tensor.matmul(
                    ps,
                    lhsT=w_sb[c][:, which * DKC + pr * P:which * DKC + (pr + 1) * P],
                    rhs=xT[c][:, sb * 512:(sb + 1) * 512],
                    start=(c == 0), stop=(c == NDC - 1))
                if c == 2:
                    yield None
            nc.vector.tensor_scalar(
                out=dst[:, sb * 512:(sb + 1) * 512], in0=ps,
                scalar1=bias[:, pr:pr + 1], scalar2=None, op0=ADD)
            yield (f"q{pr}_{sb}" if which == 0 else f"k{pr}_{sb}")

    def g_proj(sb):
        for i in range(4):
            st = sb * 4 + i
            ostg = outst.tile([P, D], F32, name="ostg", tag="ostg")
            for half in range(2):
                pp = ps_misc.tile([P, 512], F32, name="pp", tag="misc")
                for t3 in range(NPAIR):
                    nc.tensor.matmul(
                        pp[:, 0:384],
                        lhsT=attnT[t3][:, st * P:(st + 1) * P],
                        rhs=wproj_sb[t3][:, half * 384:(half + 1) * 384],
                        start=(t3 == 0), stop=(t3 == 2))
                yield None
                nc.vector.tensor_copy(out=ostg[:, half * 384:(half + 1) * 384],
                                      in_=pp[:, 0:384])
                yield None
            nc.sync.dma_start(out=out_d[st * P:(st + 1) * P, :], in_=ostg)
            yield None

    fq = FillQ()
    # upfront (cold clock, DMA-paced): only what attention chunk 0 needs --
    # kT(pair0) sb-chunk 0, qT(pair0, sb0), v chunks 0-5. The rest fills in.
    fq.drive_all(g_qk(0, 1, [0]))
    fq.drive_all(g_qk(0, 0, [0]))
    fq.drive_all(g_v(range(0, 6)))
    # paced fillers, ordered by first-need
    fq.push(g_v(range(6, 16)))
    fq.push(g_qk(0, 1, [1, 2, 3]))
    fq.push(g_qk(1, 1, [0, 1, 2, 3]))
    fq.push(g_qk(1, 0, [0]))
    fq.push(g_qk(0, 0, [1]))
    fq.push(g_qk(2, 1, [0, 1, 2, 3]))
    fq.push(g_qk(2, 0, [0]))
    fq.push(g_qk(0, 0, [2]))
    fq.push(g_qk(1, 0, [1]))
    fq.push(g_qk(2, 0, [1]))
    fq.push(g_qk(0, 0, [3]))
    fq.push(g_qk(1, 0, [2]))
    fq.push(g_qk(2, 0, [2]))
    fq.push(g_qk(1, 0, [3]))
    fq.push(g_qk(2, 0, [3]))

    # --- attention ---------------------------------------------------------
    # Score psum tiles are per sk-chunk and hold BOTH heads side by side
    # ([:, 0:512] = head A, [:, 512:1024] = head B, written by one
    # row-tiled concurrent matmul pair). One exp covers the pair (FD=1024).
    # With bufs=2 this gives a full chunk of lookahead: scores(ck+1) fill
    # while exp(ck) runs, so the ACT stream never waits on the PE refill.
    def emit_av(pr, prev, accA, accB):
        ck, e = prev
        st_f = (ck == 0)
        sp_f = (ck == NST - 1)
        nc.tensor.matmul(accA, lhsT=vaug[:, 2 * pr, ck, :],
                         rhs=e[:, 0:512], start=st_f, stop=sp_f)
        nc.tensor.matmul(accB, lhsT=vaug[:, 2 * pr + 1, ck, :],
                         rhs=e[:, 512:1024], start=st_f, stop=sp_f)

    # Deferred-normalize bookkeeping: the 2x3.3us DVE reciprocal burst for a
    # finished pair is emitted at chunk 1 of the NEXT pair -- a point where no
    # filler psum-ring consumer can queue behind it on the in-order DVE (the
    # boundary-stall / HAM-throttle trigger seen in traces).
    pending_norm = []

    def drain_norm():
        while pending_norm:
            pr_, sb_, stg_ = pending_norm.pop(0)
            for hh, st_t in enumerate(stg_):
                rec = smalls.tile([64, 512], F32, name="rec", tag="rec")
                nc.vector.reciprocal(out=rec, in_=st_t[64:128, :])
                nc.vector.tensor_tensor(
                    out=attnT[pr_][64 * hh:64 * (hh + 1),
                                   sb_ * 512:(sb_ + 1) * 512],
                    in0=st_t[0:64, :], in1=rec, op=MULT)

    for sb in range(NSQ):
        for pr in range(NPAIR):
            fq.flush_until(f"q{pr}_{sb}")
            accA = ps_acc.tile([P, 512], F32, name="accA", tag="acc")
            accB = ps_acc.tile([P, 512], F32, name="accB", tag="acc")
            prevs = []
            for ck in range(NST):
                fq.flush_until(f"k{pr}_{ck // 4}")
                fq.flush_until(f"v{ck}")
                if ck == 1:
                    drain_norm()
                # No fillers until the normalize burst has drained (ck>=8);
                # their misc-psum slot consumers (DVE) would queue behind it
                # and stall the in-order PE stream for many us.
                if ck >= 8 or (sb == 0 and pr == 0):
                    fq.step(2)
                sc = ps_sc.tile([P, 1024], F32, name="sc", tag="sc")
                nc.tensor.matmul(
                    sc[:, 0:512],
                    lhsT=kT[pr][0:64, ck * P:(ck + 1) * P],
                    rhs=qT[pr][0:64, sb * 512:(sb + 1) * 512],
                    start=True, stop=True, tile_position=(0, 0))
                nc.tensor.matmul(
                    sc[:, 512:1024],
                    lhsT=kT[pr][64:128, ck * P:(ck + 1) * P],
                    rhs=qT[pr][64:128, sb * 512:(sb + 1) * 512],
                    start=True, stop=True, tile_position=(64, 0))
                # attn@v runs two exps behind: the pair's first accumulator
                # write lands ~2.7us in, past the boundary DVE backlog that
                # releases the acc bank (avoids the in-order PE stall).
                if len(prevs) == 2:
                    emit_av(pr, prevs.pop(0), accA, accB)
                e = epool.tile([P, 1024], BF16, name="e", tag="e")
                nc.scalar.activation(out=e, in_=sc, func=EXP)
                prevs.append((ck, e))
            for pv in prevs:
                emit_av(pr, pv, accA, accB)

            # Copy-first normalize: a cheap FD-bound PSUM->SBUF copy frees the
            # acc bank in ~0.7us; the recip+mult burst is deferred to the
            # next pair's chunk 1 (see drain_norm above).
            stg = []
            for acc in (accA, accB):
                st_t = stage.tile([P, 512], F32, name="stg", tag="stage")
                nc.vector.tensor_copy(out=st_t, in_=acc)
                stg.append(st_t)
            pending_norm.append((pr, sb, stg))
        fq.push(g_proj(sb))
    drain_norm()
    fq.flush_all()


_NC_CACHE = None


def _get_nc():
    global _NC_CACHE
    if _NC_CACHE is None:
        _NC_CACHE = build_kernel()
    return _NC_CACHE


def make_in_maps(hidden_states, W_attn, b_attn, W_proj, b_proj):
    in_maps = []
    qscale = 1.0 / 8.0  # 1/sqrt(HD), folded into Wq/bq on host
    for c in range(8):
        b, g = c // 2, c % 2
        cols = slice(g * DKC, (g + 1) * DKC)
        wq = W_attn[:, 0 * D:1 * D][:, cols] * qscale
        wk = W_attn[:, 1 * D:2 * D][:, cols]
        wv = W_attn[:, 2 * D:3 * D][:, cols]
        bq = b_attn[0 * D:1 * D][cols] * qscale
        bk = b_attn[1 * D:2 * D][cols]
        bv = b_attn[2 * D:3 * D][cols]
        in_maps.append({
            "xT": np.ascontiguousarray(hidden_states[b].T).astype(ml_dtypes.bfloat16),
            "wqkv": np.ascontiguousarray(
                np.concatenate([wq, wk, wv], axis=1)).astype(ml_dtypes.bfloat16),
            "bqkv": np.ascontiguousarray(
                np.concatenate([bq, bk, bv]), dtype=np.float32),
            "wproj": np.ascontiguousarray(
                W_proj[g * DKC:(g + 1) * DKC, :]).astype(ml_dtypes.bfloat16),
        })
    return in_maps


def run(hidden_states, W_attn, b_attn, W_proj, b_proj, trace=False):
    nc = _get_nc()
    in_maps = make_in_maps(hidden_states, W_attn, b_attn, W_proj, b_proj)
    res = run_bass_kernel_spmd(nc, in_maps, core_ids=list(range(8)), trace=trace)
    out = np.empty((B, S, D), dtype=np.float32)
    for b in range(B):
        out[b] = res.results[2 * b]["out"] + res.results[2 * b + 1]["out"] + b_proj
    return out, res


def kernel(hidden_states, W_attn, b_attn, W_proj, b_proj):
    hidden_states = np.asarray(hidden_states, dtype=np.float32)
    W_attn = np.asarray(W_attn, dtype=np.float32)
    b_attn = np.asarray(b_attn, dtype=np.float32)
    W_proj = np.asarray(W_proj, dtype=np.float32)
    b_proj = np.asarray(b_proj, dtype=np.float32)
    out, _ = run(hidden_states, W_attn, b_attn, W_proj, b_proj, trace=False)
    return out
